# revision 30
# baseline (speedup 1.0000x reference)
"""Trainium2 Bass kernel for Gemma4Audio chunked local attention.

Sharding: 8 cores = batch(4) x seq-half(2). Each core processes 3072
tokens of one batch (plus a 12-token left halo and 4-token right pad)
fully locally -- block-local attention never crosses the half-sequence
boundary mid-block, so no collectives are needed.

Optimized pipeline:
- q/k/v projections run as fp8(e4m3) DoubleRow matmuls with hi/lo
  residual splits of both x and W (out = Xh@Wh + Xh@Wl + Xl@Wh), at
  0.75x the fp16 cycle count and near-fp16 accuracy. Operands are
  pre-scaled by powers of two into e4m3's normal range on the host;
  the scales are folded back out in the ACT-engine PSUM->SBUF copies.
- rel_k = pos_emb @ Wrel.T is precomputed on the host (tiny), dropping
  the Wrel weight DMA and per-head rel_k matmuls.
- engine balance: ACT owns the q/k/v scaled copies plus tanh/exp (so
  the scores matmuls never wait behind DVE queue junk), DVE owns the
  fp16-fast ops (mask add, probs normalize, at/vt copies) and the
  fp32 lg/reduce, Pool owns the scatter plus aoT/po copies.
- fp16 softmax working set (lg tile, exp biased by -SOFTCAP so values
  stay in fp16 range) enables DVE 2x/4x modes.
- PE order per iteration: front(it) -> transposes(fin) -> post ->
  out(fin) -> scores(it); the post projection fills the PSUM-copy
  latency between the front and scores. sall is double-buffered so
  scores(it) never waits on lg(it-1)'s read.
- x DMAs with hi/lo interleaved per token (800B descriptors), weights
  in 4-head 512B-descriptor groups, head-major so head h's slices
  stay ahead of iteration h; fp16 output DMA.
"""

import math

import numpy as np

# ---- model constants (hardcoded per problem spec) ----
HID = 1024
H = 8
D = 128
CHUNK = 12
PAST = 12
CTX = 24  # context window per block
P = 25  # relative positions
SOFTCAP = 50.0
Q_SCALE = D ** (-0.5) / math.log(2)
K_SCALE = math.log(1.0 + math.e) / math.log(2)

B = 4
S = 6144
NCORES = 8

T = S // 2  # 3072 tokens per core
THALO = T + PAST + 4  # 3088 with left halo + right pad for 112-wide windows
TR = 384  # tokens per region
NREG = T // TR  # 8
TPB = 96  # queries per attention tile (8 blocks)
NTILE = TR // TPB  # 4
WIN = 112  # key window per tile (96 + 12 band + 4 pad, masked)
W = 108  # live key columns feeding the output matmul
KC = HID // 128  # 8 contraction chunks
NP25 = 26  # padded rel-position count for scatter (even)
DFREE = 112  # scatter destination free size per tile
MASKVAL = -30000.0  # fits fp16; tanh saturates -> exp(-50) ~ 0

XSC = 16.0  # fp8 pre-scale for x (max |x| ~5.2 -> ~84, inside e4m3 range)

_CACHE = {}


def _build_tables():
    """Host-precomputed scatter index tables and band mask (batched over
    the NTILE tiles of a head-region: targets offset by g*DFREE)."""
    idx = np.full((128, NTILE * 2 * NP25), -1, dtype=np.int16)
    mask = np.full((128, NTILE * DFREE), MASKVAL, dtype=np.float16)
    for a in range(TPB):
        i, c = divmod(a, CHUNK)
        for g in range(NTILE):
            mask[a, g * DFREE + 12 * i : g * DFREE + 12 * i + CTX] = 0.0
            for p in range(P):
                # term A: own-row rel score at window col a+p (ctx col c+p)
                if c + p < CTX:
                    idx[a, g * 2 * NP25 + p] = g * DFREE + a + p
                # term B (rel_shift row leak): prev query's rel score
                if p >= P - c:
                    idx[a, g * 2 * NP25 + NP25 + p] = g * DFREE + a + p - P
    return idx, mask


def _build_bass(wscales):
    import concourse.bass as bass
    import concourse.bacc as bacc
    import concourse.mybir as mybir
    import concourse.tile as tile
    from concourse.masks import make_identity

    dt = mybir.dt
    f32 = dt.float32
    f16 = dt.float16
    f8 = dt.float8e4
    AF = mybir.ActivationFunctionType
    ADD = mybir.AluOpType.add
    SUB = mybir.AluOpType.subtract
    AXX = mybir.AxisListType.X
    DR = mybir.MatmulPerfMode.DoubleRow

    nc = bacc.Bacc(None, target_bir_lowering=False)

    # hi/lo interleaved per token: row p, col 2*t+{0(hi),1(lo)}
    xT8 = nc.declare_dram_parameter("xT8", [HID, THALO * 2], f8, isOutput=False)
    # weights hi/lo interleaved per output channel: col 2*o+{0(hi),1(lo)}
    wparams = {}
    for name in ("wq", "wk", "wv"):
        wparams[name] = nc.declare_dram_parameter(
            name + "T8", [HID, HID * 2], f8, isOutput=False
        )
    wparams["wp"] = nc.declare_dram_parameter("wpT8", [HID, HID * 2], f8, isOutput=False)
    relkT = nc.declare_dram_parameter("relkT", [128, H * 32], f16, isOutput=False)
    idxtab = nc.declare_dram_parameter("idxtab", [128, NTILE * 2 * NP25], dt.int16, isOutput=False)
    masktab = nc.declare_dram_parameter("masktab", [128, NTILE * DFREE], f16, isOutput=False)
    outT = nc.declare_dram_parameter("outT", [HID, T], f16, isOutput=True)

    with tile.TileContext(nc) as tc:
        with (
            tc.tile_pool(name="consts", bufs=1) as cpool,
            tc.tile_pool(name="pj", bufs=3, space="PSUM") as pjpool,
            tc.tile_pool(name="psS", bufs=2, space="PSUM") as pspoolS,
            tc.tile_pool(name="psB", bufs=1, space="PSUM") as pspoolB,
            tc.tile_pool(name="psT", bufs=1, space="PSUM") as pspoolT,
            tc.tile_pool(name="psO", bufs=1, space="PSUM") as pspoolO,
        ):
            idx_sb = cpool.tile([128, NTILE * 2 * NP25], dt.int16, tag="idx")
            nc.sync.dma_start(out=idx_sb[:], in_=idxtab[:, :])
            mask_sb = cpool.tile([128, NTILE * DFREE], f16, tag="mask")
            nc.sync.dma_start(out=mask_sb[:], in_=masktab[:, :])
            relk_sb = cpool.tile([128, H, 32], f16, tag="relk")
            nc.sync.dma_start(
                out=relk_sb[:], in_=relkT.rearrange("p (h c) -> p h c", h=H)
            )
            ident = cpool.tile([128, 128], f16, tag="ident")
            make_identity(nc, ident[:])
            negcap = cpool.tile([128, 1], f32, tag="negcap")
            nc.gpsimd.memset(negcap[:], -SOFTCAP)

            with (
                tc.tile_pool(name="weights", bufs=1) as wpool,
                tc.tile_pool(name="xin", bufs=2) as xpool,
                tc.tile_pool(name="strips", bufs=2) as spool,
                tc.tile_pool(name="ao", bufs=2) as aopool,
                tc.tile_pool(name="attn", bufs=2) as apool,
            ):
                w_sb = {}
                wviews = {}
                for pn, drh in wparams.items():
                    # one tile per 2-head group so a head's read depends only
                    # on its own DMA transfer (strided views defeat
                    # slice-level dependency tracking)
                    w_sb[pn] = [
                        wpool.tile([128, KC, 512], f8, tag=f"{pn}{g}", name=f"{pn}{g}")
                        for g in range(H // 2)
                    ]
                    wviews[pn] = drh.rearrange("(kc p) o -> p kc o", p=128)

                _main(nc, tc, mybir, AF, ADD, SUB, AXX, DR, w_sb, wviews, xT8,
                      outT, idx_sb, mask_sb, ident, relk_sb, negcap, xpool,
                      spool, aopool, apool, pjpool, pspoolS, pspoolB, pspoolT,
                      pspoolO, f32, f16, f8, wscales)
    nc.compile()
    return nc


def _main(nc, tc, mybir, AF, ADD, SUB, AXX, DR, w_sb, wviews, xT8, outT,
          idx_sb, mask_sb, ident, relk_sb, negcap, xpool, spool, aopool,
          apool, pjpool, pspoolS, pspoolB, pspoolT, pspoolO, f32, f16, f8,
          wscales):
    NIT = NREG * H  # 64 head-region iterations
    STAG = 3  # software-pipeline stagger (covers the softmax chain latency)

    # per-iteration live state, keyed it -> dict
    st = {}
    xr_by_reg = {}
    ao8_by_reg = {}
    marks = _CACHE.setdefault("stage_marks", [])
    marks.clear()

    def mark(label):
        marks.append((nc.next_id(), label))

    xview = xT8.rearrange("(kc p) nt -> p kc nt", p=128)

    def prefetch_xr(r):
        # [128, KC, (TR+16) tokens x {hi,lo}] -- one dma_start per region,
        # 800B descriptor lines
        xr = xpool.tile([128, KC, (TR + 16) * 2], f8, tag="xr", name="xr")
        c0 = r * TR * 2
        nc.sync.dma_start(out=xr[:], in_=xview[:, :, c0 : c0 + (TR + 16) * 2])
        # strided views: hi = [:, kc, 2*t], lo = [:, kc, 2*t+1]
        xr_by_reg[r] = xr.rearrange("p kc (n two) -> p kc n two", two=2)

    def stage_front(it):
        """fp8 DoubleRow projections for iteration `it` = (r, h):
        out = Xh@(Wh) + Xl@Wh + Xh@Wl, chunk-paired, one PSUM group each."""
        r, h = divmod(it, H)
        xr = xr_by_reg[r]
        hs = slice(h * 128, (h + 1) * 128)

        qps = pjpool.tile([128, TR + 2], f32, tag="pj")
        kps = pjpool.tile([128, TR + 16], f32, tag="pj")
        vps = pjpool.tile([128, TR + 12], f32, tag="pj")
        plan = (
            (qps, "wq", slice(10, TR + 12)),
            (kps, "wk", slice(0, TR + 16)),
            (vps, "wv", slice(0, TR + 12)),
        )
        os_ = slice((h % 2) * 128, (h % 2) * 128 + 128)
        for ps, wname, cs in plan:
            wv2 = w_sb[wname][h // 2].rearrange("p kc (o two) -> p kc o two", two=2)
            passes = ((0, 0), (1, 0), (0, 1))  # (w hi/lo, x hi/lo)
            first = True
            for pi, (wpart, xpart) in enumerate(passes):
                for kc in range(0, KC, 2):
                    nc.tensor.matmul(
                        ps[:], lhsT=wv2[:, kc : kc + 2, os_, wpart],
                        rhs=xr[:, kc : kc + 2, cs, xpart],
                        start=first, stop=(pi == 2 and kc == KC - 2),
                        perf_mode=DR,
                    )
                    first = False
        q_sb = spool.tile([128, TR + 2], f16, tag="q")
        k_sb = spool.tile([128, TR + 16], f16, tag="k")
        v_sb = spool.tile([128, TR + 12], f16, tag="v", bufs=STAG + 1)
        nc.scalar.mul(q_sb[:], qps[:], 1.0 / (XSC * wscales["wq"]))
        nc.scalar.mul(k_sb[:], kps[:], 1.0 / (XSC * wscales["wk"]))
        nc.scalar.mul(v_sb[:], vps[:], 1.0 / (XSC * wscales["wv"]))
        st[it] = dict(q=q_sb, k=k_sb, v=v_sb)

    def stage_scores(it):
        """Content + rel-position score matmuls for iteration `it`."""
        r, h = divmod(it, H)
        s = st[it]
        q_sb, k_sb = s["q"], s["k"]
        sall = pspoolS.tile([TPB, NTILE, WIN], f32, tag="sall")
        bdall = pspoolB.tile([TPB, NTILE, 2 * NP25], f32, tag="bd")
        for g in range(NTILE):
            b0 = TPB * g
            qmain = q_sb[:, b0 + 2 : b0 + 2 + TPB]
            qprev = q_sb[:, b0 + 1 : b0 + 1 + TPB]
            nc.tensor.matmul(
                sall[:, g, :], lhsT=qmain, rhs=k_sb[:, b0 : b0 + WIN],
                start=True, stop=True,
            )
            nc.tensor.matmul(
                bdall[:, g, 0:NP25], lhsT=qmain, rhs=relk_sb[:, h, 0:NP25],
                start=True, stop=True,
            )
            nc.tensor.matmul(
                bdall[:, g, NP25 : 2 * NP25], lhsT=qprev, rhs=relk_sb[:, h, 0:NP25],
                start=True, stop=True,
            )
        s["sall"], s["bdall"] = sall, bdall

    def stage_softmax(it):
        """Scatter + mask: DVE(bd copy) -> Pool(scatter) -> DVE(mask add,
        fp16 2x)."""
        s = st[it]
        data = apool.tile([TPB, NTILE, 2 * NP25], f16, tag="data")
        nc.vector.tensor_copy(data[:], s["bdall"][:])
        dst = apool.tile([TPB, NTILE * DFREE], f16, tag="dst")
        nc.gpsimd.local_scatter(
            dst[:], data[:], idx_sb[0:TPB, :],
            channels=TPB, num_elems=NTILE * DFREE, num_idxs=NTILE * 2 * NP25,
        )
        nc.vector.tensor_tensor(
            out=dst[:], in0=dst[:], in1=mask_sb[0:TPB, :], op=ADD,
        )
        s["dst"] = dst

    def stage_normalize(it):
        """Deferred softcap/exp/normalize tail, one iteration late. fp16
        working set; exp biased by -SOFTCAP to stay in fp16 range."""
        s = st[it]
        lg = apool.tile([TPB, NTILE, DFREE], f32, tag="lg")
        nc.vector.tensor_tensor(
            out=lg[:], in0=s["sall"][:],
            in1=s["dst"].rearrange("p (g w) -> p g w", g=NTILE), op=ADD,
        )
        nc.scalar.activation(out=lg[:], in_=lg[:], func=AF.Tanh, scale=1.0 / SOFTCAP)
        nc.scalar.activation(out=lg[:], in_=lg[:], func=AF.Exp, scale=SOFTCAP)
        rsum = apool.tile([TPB, NTILE], f32, tag="rsum")
        nc.vector.tensor_reduce(out=rsum[:], in_=lg[:], axis=AXX, op=ADD)
        nc.vector.reciprocal(rsum[:], rsum[:])
        # x8: pre-scale probs so the out-matmul PSUM is 8*ao, centering the
        # e4m3 hi/lo split of the attention output
        nc.vector.tensor_scalar_mul(rsum[:], rsum[:], 8.0)
        pr = apool.tile([TPB, NTILE, W], f16, tag="pr", bufs=STAG + 1)
        for g in range(NTILE):
            nc.gpsimd.tensor_scalar_mul(
                out=pr[:, g, :], in0=lg[:, g, 0:W], scalar1=rsum[:, g : g + 1],
            )
        s["pr"] = pr

    def stage_transpose(it):
        """PE transposes of probs + V for iteration `it` (ready long ago)."""
        s = st[it]
        pr, v_sb = s["pr"], s["v"]
        atvt = pspoolT.tile([W, NTILE, TPB + 128], f16, tag="atvt")
        for g in range(NTILE):
            b0 = TPB * g
            nc.tensor.transpose(
                atvt[:, g, 0:TPB], pr[:, g, :], ident[0:TPB, 0:TPB]
            )
            nc.tensor.transpose(
                atvt[:, g, TPB : TPB + 128], v_sb[:, b0 : b0 + W], ident[:, :]
            )
        s["atvt"] = atvt

    def stage_copies(it):
        """PSUM->SBUF copies of the transposed tiles (DVE, fp16 2x)."""
        s = st[it]
        at_sb = apool.tile([W, NTILE, TPB], f16, tag="at")
        vt_sb = apool.tile([W, NTILE, 128], f16, tag="vt")
        nc.vector.tensor_copy(at_sb[:], s["atvt"][:, :, 0:TPB])
        nc.vector.tensor_copy(vt_sb[:], s["atvt"][:, :, TPB : TPB + 128])
        s["at"], s["vt"] = at_sb, vt_sb

    def stage_out(it):
        """Attention-output matmuls + hi/lo e4m3 split of the (x8
        pre-scaled) attention output for the fp8 post projection."""
        r, h = divmod(it, H)
        s = st[it]
        aops = pspoolO.tile([128, NTILE, TPB], f32, tag="aops")
        for g in range(NTILE):
            nc.tensor.matmul(
                aops[:, g, :], lhsT=s["vt"][:, g, :], rhs=s["at"][:, g, :],
                start=True, stop=True,
            )
        if h == 0:
            ao8_by_reg[r] = (
                aopool.tile([128, H, TR], f8, tag="aoh", name="aoh"),
                aopool.tile([128, H, TR], f8, tag="aol", name="aol"),
            )
        aoh8, aol8 = ao8_by_reg[r]
        nc.scalar.copy(aoh8[:, h, :], aops[:])
        nc.vector.tensor_tensor(
            out=aol8[:, h, :], in0=aops[:], in1=aoh8[:, h, :], op=SUB,
        )
        del st[it]

    POSC = 1.0 / (8.0 * wscales["wp"])  # probs carry x8; wp carries wscale

    def emit_post(r, ocs):
        """fp8 DoubleRow post projection for region r: pps = AOh@(WPh+WPl)
        + AOl@WPh, head-pair chunked."""
        aoh8, aol8 = ao8_by_reg[r]
        for oc in ocs:
            wv2 = w_sb["wp"][oc // 2].rearrange("p kc (o two) -> p kc o two", two=2)
            os_ = slice((oc % 2) * 128, (oc % 2) * 128 + 128)
            pps = pjpool.tile([128, TR], f32, tag="pj")
            first = True
            for pi, (wpart, ao) in enumerate(((0, aoh8), (1, aoh8), (0, aol8))):
                for hp in range(0, H, 2):
                    nc.tensor.matmul(
                        pps[:], lhsT=wv2[:, hp : hp + 2, os_, wpart],
                        rhs=ao[:, hp : hp + 2, :],
                        start=first, stop=(pi == 2 and hp == H - 2),
                        perf_mode=DR,
                    )
                    first = False
            po = apool.tile([128, TR], f16, tag="po", bufs=4)
            if oc % 2 == 1:
                nc.vector.tensor_scalar_mul(po[:], pps[:], POSC)
            else:
                nc.scalar.mul(po[:], pps[:], POSC)
            nc.sync.dma_start(
                out=outT[oc * 128 : (oc + 1) * 128, r * TR : (r + 1) * TR],
                in_=po[:],
            )

    # ---- software-pipelined main loop (STAG-iteration stagger) ----
    # DMA issue order matters (single in-order queue): first-iteration
    # input, then weight slices in 4-head 512B-descriptor groups,
    # head-major so head h's slices stay ahead of iteration h, wp (first
    # needed at region-0 post) last.
    prefetch_xr(0)
    for h0 in range(0, H, 2):
        # interleaved hi/lo: 2-head slice = 512B descriptor lines, and one
        # transfer delivers both hi and lo (fast startup)
        hs = slice(h0 * 256, (h0 + 2) * 256)
        for name in ("wq", "wk", "wv"):
            nc.sync.dma_start(out=w_sb[name][h0 // 2][:], in_=wviews[name][:, :, hs])
    for g in range(H // 2):
        hsp = slice(g * 512, (g + 1) * 512)
        nc.sync.dma_start(out=w_sb["wp"][g][:], in_=wviews["wp"][:, :, hsp])

    def post_for(it):
        """Uniform 1-oc-per-iteration post schedule: iteration (r, h) does
        oc h-3 of region r-1 for h in 3..7, and oc 5+h of region r-2 for
        h in 0..2 (one body later than region r-1's last stage_out, since
        post is emitted before stage_out in the body). Region 7's chunks
        are flushed after the loop."""
        r, h = divmod(it, H)
        if h >= 3 and r >= 1:
            return (r - 1, h - 3)
        if h <= 2 and r >= 2:
            return (r - 2, 5 + h)
        return None

    for it in range(NIT + STAG):
        fin = it - STAG
        if it < NIT:
            mark(f"it{it}:front")
            stage_front(it)
        if fin >= 0:
            mark(f"it{it}:transpose")
            stage_transpose(fin)
            mark(f"it{it}:copies")
            stage_copies(fin)
        if it < NIT:
            pf = post_for(it)
            if pf is not None:
                mark(f"it{it}:post")
                emit_post(pf[0], [pf[1]])
        elif it - NIT < 3:
            # drain: region 6's last three chunks land here
            mark(f"it{it}:post")
            emit_post(NREG - 2, [5 + (it - NIT)])
        if 0 <= it - 1 < NIT:
            mark(f"it{it}:normalize")
            stage_normalize(it - 1)
        if it < NIT:
            mark(f"it{it}:scores")
            stage_scores(it)
        if fin >= 0:
            mark(f"it{it}:out")
            stage_out(fin)
        if it < NIT:
            r, h = divmod(it, H)
            mark(f"it{it}:softmax")
            stage_softmax(it)
            if h == 5 and r + 1 < NREG:
                mark(f"it{it}:xrpf")
                prefetch_xr(r + 1)
    mark("final_post")
    emit_post(NREG - 1, list(range(KC)))
    mark("end")


def _get_nc(wscales):
    key = tuple(sorted(wscales.items()))
    if _CACHE.get("key") != key:
        _CACHE["nc"] = _build_bass(wscales)
        _CACHE["key"] = key
    return _CACHE["nc"]


def _pow2_scale(w):
    """Power-of-two scale putting max|w| around 128 (e4m3 max is 240)."""
    m = float(np.abs(w).max())
    return float(2.0 ** np.floor(np.log2(128.0 / m)))


def _split8(a):
    """hi/lo e4m3 split of a float array (already pre-scaled)."""
    import ml_dtypes

    E4 = ml_dtypes.float8_e4m3
    hi = a.astype(np.float32).astype(E4)
    lo = (a.astype(np.float32) - hi.astype(np.float32)).astype(E4)
    return hi, lo


def _prepare_in_maps(hidden_states, position_embeddings, Wq, Wk, Wv, Wpost, Wrel,
                     per_dim_scale):
    f16 = np.float16
    hs = np.asarray(hidden_states, np.float32)
    pe = np.asarray(position_embeddings, np.float32)
    qscale = (Q_SCALE * np.log1p(np.exp(np.asarray(per_dim_scale, np.float64)))).astype(
        np.float64
    )
    qs_tiled = np.tile(qscale, H)  # per output channel o: scale[o % 128]
    wq_s = (np.asarray(Wq, np.float64) * qs_tiled[:, None]).T
    wk_s = (np.asarray(Wk, np.float64) * K_SCALE).T
    wv_s = np.asarray(Wv, np.float64).T
    wscales = {
        "wq": _pow2_scale(wq_s),
        "wk": _pow2_scale(wk_s),
        "wv": _pow2_scale(wv_s),
    }
    def _interleave(hi, lo):
        w8 = np.empty((HID, HID, 2), hi.dtype)
        w8[:, :, 0] = hi
        w8[:, :, 1] = lo
        return np.ascontiguousarray(w8.reshape(HID, HID * 2))

    wq8 = _interleave(*_split8(wq_s * wscales["wq"]))
    wk8 = _interleave(*_split8(wk_s * wscales["wk"]))
    wv8 = _interleave(*_split8(wv_s * wscales["wv"]))
    wp_s = np.asarray(Wpost, np.float64).T
    wscales["wp"] = _pow2_scale(wp_s)
    wp8 = _interleave(*_split8(wp_s * wscales["wp"]))

    # host-precomputed rel_k: [P, H, D] -> relkT [128(d), H*32]
    relk = (pe @ np.asarray(Wrel, np.float32).T).reshape(P, H, D)
    relkT = np.zeros((128, H, 32), f16)
    relkT[:, :, :P] = relk.transpose(2, 1, 0).astype(f16)
    relkT = np.ascontiguousarray(relkT.reshape(128, H * 32))

    idx, mask = _build_tables()

    shared = dict(wqT8=wq8, wkT8=wk8, wvT8=wv8,
                  wpT8=wp8, relkT=relkT, idxtab=idx, masktab=mask)
    in_maps = []
    for core in range(NCORES):
        b, half = divmod(core, 2)
        lo = half * T
        slab = np.zeros((THALO, HID), np.float32)
        src_lo = max(lo - PAST, 0)
        src_hi = min(lo + T + 4, S)
        off = src_lo - (lo - PAST)
        slab[off : off + (src_hi - src_lo), :] = hs[b, src_lo:src_hi, :]
        xh, xl = _split8(np.ascontiguousarray(slab.T) * XSC)
        x8 = np.empty((HID, THALO, 2), xh.dtype)
        x8[:, :, 0] = xh
        x8[:, :, 1] = xl
        in_maps.append(dict(xT8=np.ascontiguousarray(x8.reshape(HID, THALO * 2)),
                            **shared))
    return in_maps, wscales


def _assemble(results):
    out = np.empty((B, S, HID), np.float32)
    for core in range(NCORES):
        b, half = divmod(core, 2)
        out[b, half * T : (half + 1) * T, :] = results[core]["outT"].T.astype(np.float32)
    return out


def kernel(**inputs) -> np.ndarray:
    from concourse.bass_utils import run_bass_kernel_spmd

    in_maps, wscales = _prepare_in_maps(**inputs)
    nc = _get_nc(wscales)
    res = run_bass_kernel_spmd(nc, in_maps, list(range(NCORES)))
    return _assemble(res.results)
